# revision 9
# baseline (speedup 1.0000x reference)
"""Block-causal sparse attention (QKNorm + RoPE) for Trainium2, 8 NeuronCores.

Sharding: batch*head parallel. 2 batches x 16 heads = 32 (b,h) pairs; core c
handles batch c//4, heads 4*(c%4) .. 4*(c%4)+4. The out-projection is computed
as per-core partials over the local head channels and summed with a
ReduceScatter over each batch's 4-core group, so core c ends up owning rows
[512*(c%4), 512*(c%4)+512) of batch c//4's output.

Device pipeline per core (single Tile program):
  P1  qkv projection (x @ W_qkv.T) for the local 12 feature blocks, fp32r PE
  P2  RMSNorm (over dh=64) + RoPE on q,k in natural [token, feat] layout,
      then PE-transpose q,k into [dh, token] layout for attention
  P3  per (head, query-half, key-block): scoresT = K_j Q^T on PE, exp on ACT
      (scale=1/8 folded in; no max-subtraction -- |score/8| <= 8 since q,k are
      RMS-normalized), PV accumulation with a ones-row appended to V so the
      softmax denominator falls out of the same matmul
  P4  out-projection partials over local channels, fp32r PE
  P5  ReduceScatter(+) over the 4-core group, DMA to the external output

The block-causal mask (frames of 128 = tile size) is handled purely by loop
bounds; the single irregular exclusion (last query frame, first key frame) is
handled by zeroing those probs before the PV matmul.
"""

import numpy as np

import concourse.bass as bass
from concourse import bacc
import concourse.mybir as mybir
import concourse.tile as tile
from concourse.masks import make_identity

F32 = mybir.dt.float32
F32R = mybir.dt.float32r

B, L, D = 2, 2048, 1024
H, DH = 16, 64
TPF = 128            # tokens per frame == tile size
NT = L // 128        # 16 token tiles
HPC = 4              # heads per core
N_CORES = 8
GROUPS = [[0, 1, 2, 3], [4, 5, 6, 7]]
ROPE_THETA = 10000.0
EPS = 1e-6

# feature columns per core: [q(4*64) | k(4*64) | v(4*64)]
FQK = 512            # q+k features
FV = 256
FTOT = 768


def _mm(ap, use_f32r):
    return ap


def build_program(use_f32r=True, apply_gamma=False, qkv_bias=False):
    MMDT = F32R if use_f32r else F32
    nc = bacc.Bacc(num_devices=N_CORES)

    xT = nc.declare_dram_parameter("xT", [D, L], F32, isOutput=False)
    wq = nc.declare_dram_parameter("wq", [D, FTOT], F32, isOutput=False)
    wo = nc.declare_dram_parameter("wo", [HPC * DH, D], F32, isOutput=False)
    cosb = nc.declare_dram_parameter("cosb", [128, NT, 8, 32], F32, isOutput=False)
    sinb = nc.declare_dram_parameter("sinb", [128, NT, 8, 32], F32, isOutput=False)
    if apply_gamma:
        gam = nc.declare_dram_parameter("gam", [8, DH], F32, isOutput=False)
    if qkv_bias:
        bqk = nc.declare_dram_parameter("bqk", [FTOT], F32, isOutput=False)
    out = nc.declare_dram_parameter("out", [L // 4, D], F32, isOutput=True)

    with tile.TileContext(nc) as tc:
        with (
            tc.tile_pool(name="singles", bufs=1) as singles,
            tc.tile_pool(name="persist", bufs=1) as persist,
            tc.tile_pool(name="dram", bufs=1, space="DRAM") as dram,
        ):
            ident = singles.tile([128, 128], F32)
            make_identity(nc, ident[:])
            epst = singles.tile([128, 1], F32)
            nc.vector.memset(epst[:], EPS)

            # V (with ones column for the softmax denominator): [tok%128, tile, head, 65]
            vsb = persist.tile([128, NT, HPC, DH + 1], MMDT)
            nc.vector.memset(vsb[:, :, :, DH : DH + 1].bitcast(F32), 1.0)

            # transposed q,k head-pairs: [2*64 feat, L]
            qTs = [persist.tile([128, L], MMDT, tag=f"qTs{i}", name=f"qTs{i}") for i in range(2)]
            kTs = [persist.tile([128, L], MMDT, tag=f"kTs{i}", name=f"kTs{i}") for i in range(2)]

            if apply_gamma:
                gamt = singles.tile([128, 8, DH], F32)
                nc.sync.dma_start(
                    gamt[:],
                    bass.AP(tensor=gam.tensor, offset=gam[:].offset,
                            ap=[[0, 128]] + gam[:].ap),
                )
            if qkv_bias:
                bqkt = singles.tile([128, FTOT], F32)
                nc.sync.dma_start(
                    bqkt[:],
                    bass.AP(tensor=bqk.tensor, offset=bqk[:].offset,
                            ap=[[0, 128]] + bqk[:].ap),
                )

            # ---------------- P1 + P2 ----------------
            with (
                tc.tile_pool(name="p12sb", bufs=2) as p12,
                tc.tile_pool(name="p12w", bufs=1) as p12w,
                tc.tile_pool(name="p12ps", bufs=2, space="PSUM") as p12ps,
                tc.tile_pool(name="tps", bufs=2, space="PSUM") as tps,
            ):
                wqs = p12w.tile([128, 8, FTOT], MMDT)
                nc.gpsimd.dma_start(
                    wqs[:], wq[:].rearrange("(kc p) f -> p kc f", p=128))

                xTr = xT[:].rearrange("(kc p) l -> p kc l", p=128)

                for qtr in range(4):          # 4 token-tiles per quarter
                    t0 = qtr * 4
                    tok0 = t0 * 128

                    xq = p12.tile([128, 8, 512], MMDT, tag="xq")
                    nc.gpsimd.dma_start(xq[:], xTr[:, :, tok0 : tok0 + 512])
                    cq = p12.tile([128, 4, 8, 32], F32, tag="cq")
                    nc.sync.dma_start(cq[:], cosb[:, t0 : t0 + 4])
                    sq_ = p12.tile([128, 4, 8, 32], F32, tag="sq_")
                    nc.sync.dma_start(sq_[:], sinb[:, t0 : t0 + 4])

                    qkraw = p12.tile([128, 4, 8, DH], F32, tag="qkraw")

                    for t4 in range(4):
                        t = t0 + t4
                        qk_ps = p12ps.tile([128, FQK], F32, tag="qk_ps")
                        v_ps = p12ps.tile([128, FV], F32, tag="v_ps")
                        for kc in range(8):
                            lhsT = xq[:, kc, t4 * 128 : t4 * 128 + 128]
                            nc.tensor.matmul(
                                qk_ps[:], _mm(lhsT, use_f32r),
                                _mm(wqs[:, kc, 0:FQK], use_f32r),
                                start=(kc == 0), stop=(kc == 7))
                        for kc in range(8):
                            lhsT = xq[:, kc, t4 * 128 : t4 * 128 + 128]
                            nc.tensor.matmul(
                                v_ps[:], _mm(lhsT, use_f32r),
                                _mm(wqs[:, kc, FQK:FTOT], use_f32r),
                                start=(kc == 0), stop=(kc == 7))
                        if qkv_bias:
                            nc.vector.tensor_add(
                                qkraw[:, t4], qk_ps[:].rearrange("p (g d) -> p g d", d=DH),
                                bqkt[:, 0:FQK].rearrange("p (g d) -> p g d", d=DH))
                            nc.vector.tensor_add(
                                vsb[:, t, :, 0:DH],
                                v_ps[:].rearrange("p (g d) -> p g d", d=DH),
                                bqkt[:, FQK:FTOT].rearrange("p (g d) -> p g d", d=DH))
                        else:
                            nc.scalar.copy(
                                qkraw[:, t4],
                                qk_ps[:].rearrange("p (g d) -> p g d", d=DH))
                            nc.scalar.copy(
                                vsb[:, t, :, 0:DH],
                                v_ps[:].rearrange("p (g d) -> p g d", d=DH))

                    if apply_gamma:
                        gview = bass.AP(
                            tensor=gamt.tensor, offset=gamt[:].offset,
                            ap=[gamt[:].ap[0], [0, 4]] + gamt[:].ap[1:])
                        nc.vector.tensor_mul(qkraw[:], qkraw[:], gview)

                    # RMS statistics
                    sqt = p12.tile([128, 4, 8, DH], F32, tag="qkrot")
                    nc.vector.tensor_mul(sqt[:], qkraw[:], qkraw[:])
                    ssq = p12.tile([128, 4, 8], F32, tag="ssq")
                    nc.vector.reduce_sum(ssq[:], sqt[:], axis=mybir.AxisListType.X)
                    nc.scalar.activation(
                        ssq[:], ssq[:], mybir.ActivationFunctionType.Sqrt,
                        bias=epst[:], scale=1.0 / DH)
                    nc.vector.reciprocal(ssq[:], ssq[:])

                    # RoPE
                    qkrot = p12.tile([128, 4, 8, DH], F32, tag="qkrot")
                    q1 = qkraw[:, :, :, 0:32]
                    q2 = qkraw[:, :, :, 32:64]
                    mA = p12.tile([128, 4, 8, 32], F32, tag="mA")
                    mB = p12.tile([128, 4, 8, 32], F32, tag="mB")
                    nc.vector.tensor_mul(mA[:], q1, cq[:])
                    nc.vector.tensor_mul(mB[:], q2, sq_[:])
                    nc.vector.tensor_sub(qkrot[:, :, :, 0:32], mA[:], mB[:])
                    mC = p12.tile([128, 4, 8, 32], F32, tag="mA")
                    mD = p12.tile([128, 4, 8, 32], F32, tag="mB")
                    nc.vector.tensor_mul(mC[:], q2, cq[:])
                    nc.vector.tensor_mul(mD[:], q1, sq_[:])
                    nc.vector.tensor_add(qkrot[:, :, :, 32:64], mC[:], mD[:])

                    # apply 1/rms (broadcast [128,4,8] over dh)
                    rview = bass.AP(
                        tensor=ssq.tensor, offset=ssq[:].offset,
                        ap=ssq[:].ap + [[0, DH]])
                    nc.vector.tensor_mul(qkrot[:], qkrot[:], rview)

                    # transpose pairs into qTs/kTs
                    for t4 in range(4):
                        t = t0 + t4
                        for pr in range(4):
                            tp = tps.tile([128, 128], F32, tag="tp")
                            nc.tensor.transpose(
                                tp[:], qkrot[:, t4, 2 * pr : 2 * pr + 2, :], ident[:])
                            dst = (qTs if pr < 2 else kTs)[pr % 2]
                            nc.vector.tensor_copy(
                                dst[:, t * 128 : (t + 1) * 128], tp[:])

            # ---------------- P3: attention ----------------
            attnStack = [persist.tile([128, L], MMDT, tag=f"ast{i}", name=f"ast{i}") for i in range(2)]

            den_d = dram.tile([HPC, L], F32)
            with (
                tc.tile_pool(name="scps", bufs=2, space="PSUM") as scps,
                tc.tile_pool(name="atps", bufs=2, space="PSUM") as atps,
                tc.tile_pool(name="pbsb", bufs=3) as pbsb,
                tc.tile_pool(name="recp", bufs=2) as recp,
            ):
                for h in range(HPC):
                    kt = kTs[h // 2]
                    qt = qTs[h // 2]
                    pb0 = (h % 2) * 64
                    for half in range(2):
                        qlo = half * 1024
                        jmax = 8 if half == 0 else 16
                        at = atps.tile([DH + 1, 1024], F32, tag="at")
                        for j in range(jmax):
                            wlo = max(j * 128, qlo)       # global query col start
                            w = qlo + 1024 - wlo
                            sc = scps.tile([128, 1024], F32, tag="sc")
                            for c0 in range(0, w, 512):
                                cw = min(512, w - c0)
                                nc.tensor.matmul(
                                    sc[:, c0 : c0 + cw],
                                    _mm(kt[pb0 : pb0 + 64, j * 128 : (j + 1) * 128], use_f32r),
                                    _mm(qt[pb0 : pb0 + 64, wlo + c0 : wlo + c0 + cw], use_f32r),
                                    start=True, stop=True)
                            pb = pbsb.tile([128, 1024], MMDT, tag="pb")
                            nc.scalar.activation(
                                pb[:, 0:w], sc[:, 0:w],
                                mybir.ActivationFunctionType.Exp, scale=1.0 / 8.0)
                            if half == 1 and j == 0:
                                # mask: last query frame may not see key frame 0
                                nc.vector.memset(pb[:, 896:1024].bitcast(F32), 0.0)
                            s_rel = wlo - qlo             # window start within half
                            for b0 in range(0, 1024, 512):
                                seg0 = max(s_rel, b0)
                                seg1 = b0 + 512
                                if seg0 >= seg1:
                                    continue
                                nc.tensor.matmul(
                                    at[:, seg0:seg1],
                                    _mm(vsb[:, j, h, :], use_f32r),
                                    _mm(pb[:, seg0 - s_rel : seg1 - s_rel], use_f32r),
                                    start=(j == 0),
                                    stop=(j == jmax - 1 or (j + 1) * 128 >= qlo + seg1))
                        nc.vector.tensor_copy(
                            attnStack[h // 2][pb0 : pb0 + 64, qlo : qlo + 1024],
                            at[0:DH, :])
                        rec = recp.tile([DH + 1, 1024], F32, tag="rec")
                        nc.vector.reciprocal(rec[DH : DH + 1, :], at[DH : DH + 1, :])
                        nc.sync.dma_start(den_d[h : h + 1, qlo : qlo + 1024],
                                          rec[DH : DH + 1, :])

            # normalization: 1/den broadcast to 64 partitions via DRAM bounce
            with tc.tile_pool(name="denbp", bufs=2) as denbp:
                for pr in range(2):
                    denb = denbp.tile([128, L], F32, tag="denb")
                    for s in range(2):
                        src_ap = den_d[2 * pr + s : 2 * pr + s + 1, :]
                        nc.sync.dma_start(
                            denb[s * 64 : (s + 1) * 64, :],
                            bass.AP(tensor=src_ap.tensor, offset=src_ap.offset,
                                    ap=[[0, 64]] + src_ap.ap[1:]))
                    nc.vector.tensor_mul(
                        attnStack[pr][:], attnStack[pr][:], denb[:])

            # ---------------- P4: out projection (partial) ----------------
            part = dram.tile([L, D], F32)
            with (
                tc.tile_pool(name="wosb", bufs=1) as wosb,
                tc.tile_pool(name="osb", bufs=3) as osb,
                tc.tile_pool(name="ops", bufs=4, space="PSUM") as ops,
            ):
                wos = wosb.tile([128, 2, D], MMDT)
                nc.gpsimd.dma_start(
                    wos[:], wo[:].rearrange("(pr p) o -> p pr o", p=128))
                for tq in range(NT):
                    ost = osb.tile([128, D], F32, tag="ost")
                    for o0 in range(0, D, 512):
                        op = ops.tile([128, 512], F32, tag="op")
                        for pr in range(2):
                            nc.tensor.matmul(
                                op[:],
                                _mm(attnStack[pr][:, tq * 128 : (tq + 1) * 128], use_f32r),
                                _mm(wos[:, pr, o0 : o0 + 512], use_f32r),
                                start=(pr == 0), stop=(pr == 1))
                        nc.vector.tensor_copy(ost[:, o0 : o0 + 512], op[:])
                    nc.sync.dma_start(part[tq * 128 : (tq + 1) * 128, :], ost[:])

            # ---------------- P5: ReduceScatter + output ----------------
            rs_out = dram.tile([L // 4, D], F32)
            nc.gpsimd.collective_compute(
                "ReduceScatter", mybir.AluOpType.add,
                replica_groups=GROUPS,
                ins=[part[:].opt()], outs=[rs_out[:].opt()])
            nc.sync.dma_start(out[:], rs_out[:])

    nc.compile()
    return nc


_PROG_CACHE = {}


def _get_program(key):
    if key not in _PROG_CACHE:
        _PROG_CACHE[key] = build_program(*key)
    return _PROG_CACHE[key]


def _host_inputs(x, W_qkv, b_qkv, W_out, b_out, q_gamma, k_gamma, use_f32r=True):
    x = np.asarray(x, dtype=np.float32)
    W_qkv = np.asarray(W_qkv, dtype=np.float32)
    b_qkv = np.asarray(b_qkv, dtype=np.float32)
    W_out = np.asarray(W_out, dtype=np.float32)
    q_gamma = np.asarray(q_gamma, dtype=np.float32)
    k_gamma = np.asarray(k_gamma, dtype=np.float32)

    apply_gamma = not (np.all(q_gamma == 1.0) and np.all(k_gamma == 1.0))
    qkv_bias = bool(np.any(b_qkv))

    # rope tables: pos = t*128 + p
    pos = np.arange(L, dtype=np.float64).reshape(NT, 128).T  # [128, NT]
    inv = 1.0 / (ROPE_THETA ** (np.arange(32, dtype=np.float64) / 32.0))
    ang = pos[:, :, None] * inv[None, None, :]               # [128, NT, 32]
    cosb = np.broadcast_to(
        np.cos(ang)[:, :, None, :], (128, NT, 8, 32)).astype(np.float32).copy()
    sinb = np.broadcast_to(
        np.sin(ang)[:, :, None, :], (128, NT, 8, 32)).astype(np.float32).copy()

    Wq = W_qkv[0 * D : 1 * D]    # [1024, 1024] rows = q features
    Wk = W_qkv[1 * D : 2 * D]
    Wv = W_qkv[2 * D : 3 * D]
    WoT = np.ascontiguousarray(W_out.T)  # [d_in, d_out]

    in_maps = []
    for c in range(N_CORES):
        b = c // 4
        h0 = 4 * (c % 4)
        rows = slice(h0 * DH, (h0 + HPC) * DH)
        wq_c = np.ascontiguousarray(
            np.concatenate([Wq[rows], Wk[rows], Wv[rows]], axis=0).T)  # [1024, 768]
        wo_c = np.ascontiguousarray(WoT[rows])                         # [256, 1024]
        m = {
            "xT": np.ascontiguousarray(x[b].T),
            "wq": wq_c,
            "wo": wo_c,
            "cosb": cosb,
            "sinb": sinb,
        }
        if apply_gamma:
            m["gam"] = np.ascontiguousarray(
                np.concatenate([np.broadcast_to(q_gamma, (4, DH)),
                                np.broadcast_to(k_gamma, (4, DH))], axis=0))
        if qkv_bias:
            bq = b_qkv[0 * D : 1 * D][rows]
            bk = b_qkv[1 * D : 2 * D][rows]
            bv = b_qkv[2 * D : 3 * D][rows]
            m["bqk"] = np.ascontiguousarray(np.concatenate([bq, bk, bv]))
        in_maps.append(m)

    key = (use_f32r, apply_gamma, qkv_bias)
    return key, in_maps


def _assemble(results, b_out):
    y = np.empty((B, L, D), dtype=np.float32)
    for c in range(N_CORES):
        b = c // 4
        r = c % 4
        y[b, r * 512 : (r + 1) * 512, :] = results[c]["out"]
    b_out = np.asarray(b_out, dtype=np.float32)
    if np.any(b_out):
        y += b_out
    return y


def _install_ntff_hook():
    """Register the axon NTFF profiling hook (the container's antenv stub
    lacks axon_hooks; replicate what trn_boot would have registered)."""
    import sys
    import types
    try:
        from antenv.axon_hooks import get_axon_ntff_profile_hook  # noqa: F401
        return
    except ImportError:
        pass
    try:
        from trn_agent_boot.trn_boot import _ntff_profile_via_ctypes
        hook = _ntff_profile_via_ctypes("/opt/axon/libaxon_pjrt.so")
    except Exception:
        hook = None
    import antenv
    mod = types.ModuleType("antenv.axon_hooks")
    mod.get_axon_ntff_profile_hook = lambda: hook
    mod.set_axon_ntff_profile_hook = lambda h: None
    antenv.axon_hooks = mod
    sys.modules["antenv.axon_hooks"] = mod


def kernel(x, W_qkv, b_qkv, W_out, b_out, q_gamma, k_gamma, _trace=False):
    from concourse.bass_utils import run_bass_kernel_spmd
    if _trace:
        _install_ntff_hook()

    use_f32r = True
    key, in_maps = _host_inputs(x, W_qkv, b_qkv, W_out, b_out,
                                q_gamma, k_gamma, use_f32r)
    nc = _get_program(key)
    res = run_bass_kernel_spmd(nc, in_maps, core_ids=list(range(N_CORES)),
                               trace=_trace,
                               trace_cores=list(range(N_CORES)) if _trace else None)
    y = _assemble(res.results, b_out)
    if _trace:
        return y, res
    return y


# revision 12
# speedup vs baseline: 1.1966x; 1.1966x over previous
"""Block-causal sparse attention (QKNorm + RoPE) for Trainium2, 8 NeuronCores.

Sharding: batch*head parallel. 2 batches x 16 heads = 32 (b,h) pairs; core c
handles batch c//4, heads 4*(c%4) .. 4*(c%4)+4. The out-projection is computed
as per-core partials over the local head channels and summed with ReduceScatter
over each batch's 4-core group (chunked by query-half so the collective
overlaps the second half's attention compute).

Device pipeline per core (single Tile program):
  P1  qkv projection (x @ W_qkv.T) for the local 12 feature blocks (PE)
  P2  RMSNorm (over dh=64) + RoPE on q,k in natural [token, feat] layout,
      then PE-transpose q,k into [dh, token] layout
  P3  per (query-half, head, key-block): scoresT = K_j Q^T on PE, exp on ACT
      (scale=1/8 folded in; no max-subtraction -- |score/8| <= 8 because q,k
      are RMS-normalized), PV accumulation with a ones-row appended to V so
      the softmax denominator falls out of the same matmul
  P4  (per half) normalize by 1/den, out-projection partials, ReduceScatter

The block-causal mask (frames of 128 = tile size) is handled by loop bounds;
the single irregular exclusion (last query frame, first key frame) is handled
by zeroing those probs before the PV matmul.

Matmul operands are cast to MM_DTYPE (bf16 by default: fp32/fp32r matmuls run
2-3x slower per row and their 4-byte LDWEIGHTS can't use fast-weight-load).
Accumulation stays fp32 in PSUM; softmax/statistics math stays fp32.
"""

import numpy as np

import concourse.bass as bass
from concourse import bacc
import concourse.mybir as mybir
import concourse.tile as tile
from concourse.masks import make_identity

F32 = mybir.dt.float32
F32R = mybir.dt.float32r
BF16 = mybir.dt.bfloat16

B, L, D = 2, 2048, 1024
H, DH = 16, 64
NT = L // 128        # 16 token tiles
HPC = 4              # heads per core
N_CORES = 8
GROUPS = [[0, 1, 2, 3], [4, 5, 6, 7]]
ROPE_THETA = 10000.0
EPS = 1e-6

FQK = 512            # q+k feature columns per core
FV = 256
FTOT = 768

MM_DTYPES = {"bf16": BF16, "f32r": F32R, "f32": F32}


def build_program(mm_dtype="bf16", apply_gamma=False, qkv_bias=False):
    MMDT = MM_DTYPES[mm_dtype]
    nc = bacc.Bacc(num_devices=N_CORES)

    xT = nc.declare_dram_parameter("xT", [D, L], F32, isOutput=False)
    wq = nc.declare_dram_parameter("wq", [D, FTOT], F32, isOutput=False)
    wo = nc.declare_dram_parameter("wo", [HPC * DH, D], F32, isOutput=False)
    cosb = nc.declare_dram_parameter("cosb", [128, NT, 8, 32], F32, isOutput=False)
    sinb = nc.declare_dram_parameter("sinb", [128, NT, 8, 32], F32, isOutput=False)
    if apply_gamma:
        gam = nc.declare_dram_parameter("gam", [8, DH], F32, isOutput=False)
    if qkv_bias:
        bqk = nc.declare_dram_parameter("bqk", [FTOT], F32, isOutput=False)
    # rows [256*half + r] = my shard of query rows [1024*half + 256*rank + r]
    out = nc.declare_dram_parameter("out", [L // 4, D], F32, isOutput=True)

    with tile.TileContext(nc) as tc:
        with (
            tc.tile_pool(name="singles", bufs=1) as singles,
            tc.tile_pool(name="persist", bufs=1) as persist,
            tc.tile_pool(name="dram", bufs=1, space="DRAM") as dram,
        ):
            ident = singles.tile([128, 128], F32)
            make_identity(nc, ident[:])
            epst = singles.tile([128, 1], F32)
            nc.vector.memset(epst[:], EPS)

            # V with ones column appended: [tok%128, tile, head, 65]
            vsb = persist.tile([128, NT, HPC, DH + 1], MMDT)
            ones_ap = vsb[:, :, :, DH : DH + 1]
            if MMDT == F32R:
                ones_ap = ones_ap.bitcast(F32)
            nc.vector.memset(ones_ap, 1.0)

            # transposed q,k head-pairs: [2*64 feat, L]
            qTs = [persist.tile([128, L], MMDT, tag=f"qTs{i}", name=f"qTs{i}")
                   for i in range(2)]
            kTs = [persist.tile([128, L], MMDT, tag=f"kTs{i}", name=f"kTs{i}")
                   for i in range(2)]

            if apply_gamma:
                gamt = singles.tile([128, 8, DH], F32)
                nc.sync.dma_start(
                    gamt[:],
                    bass.AP(tensor=gam.tensor, offset=gam[:].offset,
                            ap=[[0, 128]] + gam[:].ap))
            if qkv_bias:
                bqkt = singles.tile([128, FTOT], F32)
                nc.sync.dma_start(
                    bqkt[:],
                    bass.AP(tensor=bqk.tensor, offset=bqk[:].offset,
                            ap=[[0, 128]] + bqk[:].ap))

            # ---------------- P1 + P2 ----------------
            with (
                tc.tile_pool(name="p12sb", bufs=2) as p12,
                tc.tile_pool(name="p12w", bufs=1) as p12w,
                tc.tile_pool(name="p12ps", bufs=2, space="PSUM") as p12ps,
                tc.tile_pool(name="tps", bufs=2, space="PSUM") as tps,
            ):
                wqs = p12w.tile([128, 8, FTOT], MMDT)
                nc.gpsimd.dma_start(
                    wqs[:], wq[:].rearrange("(kc p) f -> p kc f", p=128))

                xTr = xT[:].rearrange("(kc p) l -> p kc l", p=128)

                for qtr in range(4):          # 4 token-tiles per quarter
                    t0 = qtr * 4
                    tok0 = t0 * 128

                    xq = p12.tile([128, 8, 512], MMDT, tag="xq")
                    nc.gpsimd.dma_start(xq[:], xTr[:, :, tok0 : tok0 + 512])
                    cq = p12.tile([128, 4, 8, 32], F32, tag="cq")
                    nc.sync.dma_start(cq[:], cosb[:, t0 : t0 + 4])
                    sq_ = p12.tile([128, 4, 8, 32], F32, tag="sq_")
                    nc.sync.dma_start(sq_[:], sinb[:, t0 : t0 + 4])

                    qkraw = p12.tile([128, 4, 8, DH], F32, tag="qkraw")

                    for t4 in range(4):
                        t = t0 + t4
                        qk_ps = p12ps.tile([128, FQK], F32, tag="qk_ps")
                        v_ps = p12ps.tile([128, FV], F32, tag="v_ps")
                        for kc in range(8):
                            lhsT = xq[:, kc, t4 * 128 : t4 * 128 + 128]
                            nc.tensor.matmul(
                                qk_ps[:], lhsT, wqs[:, kc, 0:FQK],
                                start=(kc == 0), stop=(kc == 7))
                        for kc in range(8):
                            lhsT = xq[:, kc, t4 * 128 : t4 * 128 + 128]
                            nc.tensor.matmul(
                                v_ps[:], lhsT, wqs[:, kc, FQK:FTOT],
                                start=(kc == 0), stop=(kc == 7))
                        if qkv_bias:
                            nc.vector.tensor_add(
                                qkraw[:, t4],
                                qk_ps[:].rearrange("p (g d) -> p g d", d=DH),
                                bqkt[:, 0:FQK].rearrange("p (g d) -> p g d", d=DH))
                            nc.vector.tensor_add(
                                vsb[:, t, :, 0:DH],
                                v_ps[:].rearrange("p (g d) -> p g d", d=DH),
                                bqkt[:, FQK:FTOT].rearrange("p (g d) -> p g d", d=DH))
                        else:
                            nc.scalar.copy(
                                qkraw[:, t4],
                                qk_ps[:].rearrange("p (g d) -> p g d", d=DH))
                            nc.scalar.copy(
                                vsb[:, t, :, 0:DH],
                                v_ps[:].rearrange("p (g d) -> p g d", d=DH))

                    if apply_gamma:
                        gview = bass.AP(
                            tensor=gamt.tensor, offset=gamt[:].offset,
                            ap=[gamt[:].ap[0], [0, 4]] + gamt[:].ap[1:])
                        nc.vector.tensor_mul(qkraw[:], qkraw[:], gview)

                    # RMS statistics
                    sqt = p12.tile([128, 4, 8, DH], F32, tag="qkrot")
                    nc.vector.tensor_mul(sqt[:], qkraw[:], qkraw[:])
                    ssq = p12.tile([128, 4, 8], F32, tag="ssq")
                    nc.vector.reduce_sum(ssq[:], sqt[:], axis=mybir.AxisListType.X)
                    nc.scalar.activation(
                        ssq[:], ssq[:], mybir.ActivationFunctionType.Sqrt,
                        bias=epst[:], scale=1.0 / DH)
                    nc.vector.reciprocal(ssq[:], ssq[:])

                    # RoPE
                    qkrot = p12.tile([128, 4, 8, DH], F32, tag="qkrot")
                    q1 = qkraw[:, :, :, 0:32]
                    q2 = qkraw[:, :, :, 32:64]
                    mA = p12.tile([128, 4, 8, 32], F32, tag="mA")
                    mB = p12.tile([128, 4, 8, 32], F32, tag="mB")
                    nc.vector.tensor_mul(mA[:], q1, cq[:])
                    nc.vector.tensor_mul(mB[:], q2, sq_[:])
                    nc.vector.tensor_sub(qkrot[:, :, :, 0:32], mA[:], mB[:])
                    mC = p12.tile([128, 4, 8, 32], F32, tag="mA")
                    mD = p12.tile([128, 4, 8, 32], F32, tag="mB")
                    nc.vector.tensor_mul(mC[:], q2, cq[:])
                    nc.vector.tensor_mul(mD[:], q1, sq_[:])
                    nc.vector.tensor_add(qkrot[:, :, :, 32:64], mC[:], mD[:])

                    # apply 1/rms (broadcast [128,4,8] over dh)
                    rview = bass.AP(
                        tensor=ssq.tensor, offset=ssq[:].offset,
                        ap=ssq[:].ap + [[0, DH]])
                    nc.vector.tensor_mul(qkrot[:], qkrot[:], rview)

                    # transpose pairs into qTs/kTs (f32 -> psum, cast on copy)
                    for t4 in range(4):
                        t = t0 + t4
                        for pr in range(4):
                            tp = tps.tile([128, 128], F32, tag="tp")
                            nc.tensor.transpose(
                                tp[:], qkrot[:, t4, 2 * pr : 2 * pr + 2, :],
                                ident[:])
                            dst = (qTs if pr < 2 else kTs)[pr % 2]
                            nc.vector.tensor_copy(
                                dst[:, t * 128 : (t + 1) * 128], tp[:])

            # ---------------- P3 + P4, per query half ----------------
            attnStack = [persist.tile([128, L], MMDT, tag=f"ast{i}", name=f"ast{i}")
                         for i in range(2)]
            den_d = dram.tile([2, HPC, 1024], F32)      # [half, head, q]
            rden_d = dram.tile([2, HPC, 1024], F32)     # reciprocals
            part = dram.tile([L, D], F32)
            rs_out = dram.tile([L // 4, D], F32)
            wos = persist.tile([128, 2, D], MMDT)
            nc.gpsimd.dma_start(
                wos[:], wo[:].rearrange("(pr p) o -> p pr o", p=128))

            for half in range(2):
                qlo = half * 1024
                jmax = 8 if half == 0 else 16
                with (
                    tc.tile_pool(name=f"scps{half}", bufs=2, space="PSUM") as scps,
                    tc.tile_pool(name=f"atps{half}", bufs=2, space="PSUM") as atps,
                    tc.tile_pool(name=f"pbsb{half}", bufs=3) as pbsb,
                    tc.tile_pool(name=f"recp{half}", bufs=2) as recp,
                ):
                    for h in range(HPC):
                        kt = kTs[h // 2]
                        qt = qTs[h // 2]
                        pb0 = (h % 2) * 64
                        at = atps.tile([DH + 1, 1024], F32, tag="at")
                        for j in range(jmax):
                            wlo = max(j * 128, qlo)     # global query col start
                            w = qlo + 1024 - wlo
                            sc = scps.tile([128, 1024], F32, tag="sc")
                            for c0 in range(0, w, 512):
                                cw = min(512, w - c0)
                                nc.tensor.matmul(
                                    sc[:, c0 : c0 + cw],
                                    kt[pb0 : pb0 + 64, j * 128 : (j + 1) * 128],
                                    qt[pb0 : pb0 + 64, wlo + c0 : wlo + c0 + cw],
                                    start=True, stop=True)
                            pb = pbsb.tile([128, 1024], MMDT, tag="pb")
                            nc.scalar.activation(
                                pb[:, 0:w], sc[:, 0:w],
                                mybir.ActivationFunctionType.Exp, scale=1.0 / 8.0)
                            if half == 1 and j == 0:
                                # mask: last query frame can't see key frame 0
                                mask_ap = pb[:, 896:1024]
                                if MMDT == F32R:
                                    mask_ap = mask_ap.bitcast(F32)
                                nc.vector.memset(mask_ap, 0.0)
                            s_rel = wlo - qlo           # window start within half
                            for b0 in range(0, 1024, 512):
                                seg0 = max(s_rel, b0)
                                seg1 = b0 + 512
                                if seg0 >= seg1:
                                    continue
                                nc.tensor.matmul(
                                    at[:, seg0:seg1],
                                    vsb[:, j, h, :],
                                    pb[:, seg0 - s_rel : seg1 - s_rel],
                                    start=(j == 0),
                                    stop=(j == jmax - 1
                                          or (j + 1) * 128 >= qlo + seg1))
                        nc.vector.tensor_copy(
                            attnStack[h // 2][pb0 : pb0 + 64, qlo : qlo + 1024],
                            at[0:DH, :])
                        # stash the (un-reciprocated) denominator row
                        rec = recp.tile([DH + 1, 1024], F32, tag="rec")
                        nc.vector.tensor_copy(rec[DH : DH + 1, :],
                                              at[DH : DH + 1, :])
                        nc.sync.dma_start(den_d[half, h, :],
                                          rec[DH : DH + 1, :])

                # normalize + out-projection for this half (P3 psum pools
                # are closed so the out-proj psum pool has banks to use)
                with (
                    tc.tile_pool(name=f"denb{half}", bufs=2) as denbp,
                    tc.tile_pool(name=f"osb{half}", bufs=3) as osb,
                    tc.tile_pool(name=f"ops{half}", bufs=4, space="PSUM") as ops,
                ):
                    # reciprocal on a [128, 32] repack (single-lane DVE
                    # reciprocal on [1,1024] rows costs ~6.5us each)
                    dpak = denbp.tile([128, 32], F32, tag="dpak")
                    dh_ap = den_d[half]
                    nc.sync.dma_start(
                        dpak[:],
                        bass.AP(tensor=dh_ap.tensor, offset=dh_ap.offset,
                                ap=[[32, 128], [1, 32]]))
                    nc.vector.reciprocal(dpak[:], dpak[:])
                    rh_ap = rden_d[half]
                    nc.sync.dma_start(
                        bass.AP(tensor=rh_ap.tensor, offset=rh_ap.offset,
                                ap=[[32, 128], [1, 32]]),
                        dpak[:])

                    if True:
                        for pr in range(2):
                            denb = denbp.tile([128, 1024], F32, tag="denb")
                            for s in range(2):
                                src_ap = rden_d[half, 2 * pr + s, :]
                                nc.sync.dma_start(
                                    denb[s * 64 : (s + 1) * 64, :],
                                    bass.AP(tensor=src_ap.tensor,
                                            offset=src_ap.offset,
                                            ap=[[0, 64]] + src_ap.ap))
                            nc.vector.tensor_mul(
                                attnStack[pr][:, qlo : qlo + 1024],
                                attnStack[pr][:, qlo : qlo + 1024],
                                denb[:])
                        for tq in range(8 * half, 8 * half + 8):
                            ost = osb.tile([128, D], F32, tag="ost")
                            for o0 in range(0, D, 512):
                                op = ops.tile([128, 512], F32, tag="op")
                                for pr in range(2):
                                    nc.tensor.matmul(
                                        op[:],
                                        attnStack[pr][:, tq * 128 : (tq + 1) * 128],
                                        wos[:, pr, o0 : o0 + 512],
                                        start=(pr == 0), stop=(pr == 1))
                                nc.vector.tensor_copy(ost[:, o0 : o0 + 512], op[:])
                            nc.sync.dma_start(
                                part[tq * 128 : (tq + 1) * 128, :], ost[:])

                # ReduceScatter for this half's 1024 query rows
                nc.gpsimd.collective_compute(
                    "ReduceScatter", mybir.AluOpType.add,
                    replica_groups=GROUPS,
                    ins=[part[qlo : qlo + 1024, :].opt()],
                    outs=[rs_out[half * 256 : half * 256 + 256, :].opt()])

            nc.sync.dma_start(out[:], rs_out[:])

    nc.compile()
    return nc


_PROG_CACHE = {}


def _get_program(key):
    if key not in _PROG_CACHE:
        _PROG_CACHE[key] = build_program(*key)
    return _PROG_CACHE[key]


def _host_inputs(x, W_qkv, b_qkv, W_out, b_out, q_gamma, k_gamma,
                 mm_dtype="bf16"):
    x = np.asarray(x, dtype=np.float32)
    W_qkv = np.asarray(W_qkv, dtype=np.float32)
    b_qkv = np.asarray(b_qkv, dtype=np.float32)
    W_out = np.asarray(W_out, dtype=np.float32)
    q_gamma = np.asarray(q_gamma, dtype=np.float32)
    k_gamma = np.asarray(k_gamma, dtype=np.float32)

    apply_gamma = not (np.all(q_gamma == 1.0) and np.all(k_gamma == 1.0))
    qkv_bias = bool(np.any(b_qkv))

    # rope tables: pos = t*128 + p
    pos = np.arange(L, dtype=np.float64).reshape(NT, 128).T  # [128, NT]
    inv = 1.0 / (ROPE_THETA ** (np.arange(32, dtype=np.float64) / 32.0))
    ang = pos[:, :, None] * inv[None, None, :]               # [128, NT, 32]
    cosb = np.broadcast_to(
        np.cos(ang)[:, :, None, :], (128, NT, 8, 32)).astype(np.float32).copy()
    sinb = np.broadcast_to(
        np.sin(ang)[:, :, None, :], (128, NT, 8, 32)).astype(np.float32).copy()

    Wq = W_qkv[0 * D : 1 * D]
    Wk = W_qkv[1 * D : 2 * D]
    Wv = W_qkv[2 * D : 3 * D]
    WoT = np.ascontiguousarray(W_out.T)  # [d_in, d_out]

    in_maps = []
    for c in range(N_CORES):
        b = c // 4
        h0 = 4 * (c % 4)
        rows = slice(h0 * DH, (h0 + HPC) * DH)
        wq_c = np.ascontiguousarray(
            np.concatenate([Wq[rows], Wk[rows], Wv[rows]], axis=0).T)
        wo_c = np.ascontiguousarray(WoT[rows])
        m = {
            "xT": np.ascontiguousarray(x[b].T),
            "wq": wq_c,
            "wo": wo_c,
            "cosb": cosb,
            "sinb": sinb,
        }
        if apply_gamma:
            m["gam"] = np.ascontiguousarray(
                np.concatenate([np.broadcast_to(q_gamma, (4, DH)),
                                np.broadcast_to(k_gamma, (4, DH))], axis=0))
        if qkv_bias:
            m["bqk"] = np.ascontiguousarray(np.concatenate(
                [b_qkv[0 * D : 1 * D][rows], b_qkv[1 * D : 2 * D][rows],
                 b_qkv[2 * D : 3 * D][rows]]))
        in_maps.append(m)

    key = (mm_dtype, apply_gamma, qkv_bias)
    return key, in_maps


def _assemble(results, b_out):
    y = np.empty((B, L, D), dtype=np.float32)
    for c in range(N_CORES):
        b = c // 4
        r = c % 4
        o = results[c]["out"]
        for half in range(2):
            rows = slice(1024 * half + 256 * r, 1024 * half + 256 * r + 256)
            y[b, rows, :] = o[256 * half : 256 * half + 256]
    b_out = np.asarray(b_out, dtype=np.float32)
    if np.any(b_out):
        y += b_out
    return y


def _install_ntff_hook():
    """Register the axon NTFF profiling hook (the container's antenv stub
    lacks axon_hooks; replicate what trn_boot would have registered)."""
    import sys
    import types
    try:
        from antenv.axon_hooks import get_axon_ntff_profile_hook  # noqa: F401
        return
    except ImportError:
        pass
    try:
        from trn_agent_boot.trn_boot import _ntff_profile_via_ctypes
        hook = _ntff_profile_via_ctypes("/opt/axon/libaxon_pjrt.so")
    except Exception:
        hook = None
    import antenv
    mod = types.ModuleType("antenv.axon_hooks")
    mod.get_axon_ntff_profile_hook = lambda: hook
    mod.set_axon_ntff_profile_hook = lambda h: None
    antenv.axon_hooks = mod
    sys.modules["antenv.axon_hooks"] = mod


def kernel(x, W_qkv, b_qkv, W_out, b_out, q_gamma, k_gamma, _trace=False,
           _mm_dtype="bf16"):
    from concourse.bass_utils import run_bass_kernel_spmd
    if _trace:
        _install_ntff_hook()

    key, in_maps = _host_inputs(x, W_qkv, b_qkv, W_out, b_out,
                                q_gamma, k_gamma, _mm_dtype)
    nc = _get_program(key)
    res = run_bass_kernel_spmd(nc, in_maps, core_ids=list(range(N_CORES)),
                               trace=_trace,
                               trace_cores=list(range(N_CORES)) if _trace else None)
    y = _assemble(res.results, b_out)
    if _trace:
        return y, res
    return y


# revision 13
# speedup vs baseline: 1.4428x; 1.2058x over previous
"""Block-causal sparse attention (QKNorm + RoPE) for Trainium2, 8 NeuronCores.

Sharding: batch*head parallel. 2 batches x 16 heads = 32 (b,h) pairs; core c
handles batch c//4, heads 4*(c%4) .. 4*(c%4)+4. The out-projection is computed
as per-core partials over the local head channels and summed with ReduceScatter
over each batch's 4-core group (chunked by query-half so the collective
overlaps the second half's attention compute).

Device pipeline per core (single Tile program):
  P1  qkv projection (x @ W_qkv.T) for the local 12 feature blocks (PE)
  P2  RMSNorm (over dh=64) + RoPE on q,k in natural [token, feat] layout,
      then PE-transpose q,k into [dh, token] layout
  P3  per (query-half, head, key-block): scoresT = K_j Q^T on PE, exp on ACT
      (scale=1/8 folded in; no max-subtraction -- |score/8| <= 8 because q,k
      are RMS-normalized), PV accumulation with a ones-row appended to V so
      the softmax denominator falls out of the same matmul
  P4  (per half) normalize by 1/den, out-projection partials, ReduceScatter

The block-causal mask (frames of 128 = tile size) is handled by loop bounds;
the single irregular exclusion (last query frame, first key frame) is handled
by zeroing those probs before the PV matmul.

Matmul operands are cast to MM_DTYPE (bf16 by default: fp32/fp32r matmuls run
2-3x slower per row and their 4-byte LDWEIGHTS can't use fast-weight-load).
Accumulation stays fp32 in PSUM; softmax/statistics math stays fp32.
"""

import numpy as np

import concourse.bass as bass
from concourse import bacc
import concourse.mybir as mybir
import concourse.tile as tile
from concourse.masks import make_identity

F32 = mybir.dt.float32
F32R = mybir.dt.float32r
BF16 = mybir.dt.bfloat16

B, L, D = 2, 2048, 1024
H, DH = 16, 64
NT = L // 128        # 16 token tiles
HPC = 4              # heads per core
N_CORES = 8
GROUPS = [[0, 1, 2, 3], [4, 5, 6, 7]]
ROPE_THETA = 10000.0
EPS = 1e-6

FQK = 512            # q+k feature columns per core
FV = 256
FTOT = 768

MM_DTYPES = {"bf16": BF16, "f32r": F32R, "f32": F32}


def build_program(mm_dtype="bf16", apply_gamma=False, qkv_bias=False):
    MMDT = MM_DTYPES[mm_dtype]
    nc = bacc.Bacc(num_devices=N_CORES)

    xT = nc.declare_dram_parameter("xT", [D, L], F32, isOutput=False)
    wq = nc.declare_dram_parameter("wq", [D, FTOT], F32, isOutput=False)
    wo = nc.declare_dram_parameter("wo", [D, D], F32, isOutput=False)
    qoff = nc.declare_dram_parameter("qoff", [1, 1], mybir.dt.uint32, isOutput=False)
    cosb = nc.declare_dram_parameter("cosb", [128, NT, 8, 32], F32, isOutput=False)
    sinb = nc.declare_dram_parameter("sinb", [128, NT, 8, 32], F32, isOutput=False)
    if apply_gamma:
        gam = nc.declare_dram_parameter("gam", [8, DH], F32, isOutput=False)
    if qkv_bias:
        bqk = nc.declare_dram_parameter("bqk", [FTOT], F32, isOutput=False)
    # rows [256*half + r] = my shard of query rows [1024*half + 256*rank + r]
    out = nc.declare_dram_parameter("out", [L // 4, D], F32, isOutput=True)

    with tile.TileContext(nc) as tc:
        with (
            tc.tile_pool(name="singles", bufs=1) as singles,
            tc.tile_pool(name="persist", bufs=1) as persist,
            tc.tile_pool(name="dram", bufs=1, space="DRAM") as dram,
        ):
            ident = singles.tile([128, 128], F32)
            make_identity(nc, ident[:])
            epst = singles.tile([128, 1], F32)
            nc.vector.memset(epst[:], EPS)

            # V with ones column appended: [tok%128, tile, head, 65]
            vsb = persist.tile([128, NT, HPC, DH + 1], MMDT)
            ones_ap = vsb[:, :, :, DH : DH + 1]
            if MMDT == F32R:
                ones_ap = ones_ap.bitcast(F32)
            nc.vector.memset(ones_ap, 1.0)

            # transposed q,k head-pairs: [2*64 feat, L]
            qTs = [persist.tile([128, L], MMDT, tag=f"qTs{i}", name=f"qTs{i}")
                   for i in range(2)]
            kTs = [persist.tile([128, L], MMDT, tag=f"kTs{i}", name=f"kTs{i}")
                   for i in range(2)]

            if apply_gamma:
                gamt = singles.tile([128, 8, DH], F32)
                nc.sync.dma_start(
                    gamt[:],
                    bass.AP(tensor=gam.tensor, offset=gam[:].offset,
                            ap=[[0, 128]] + gam[:].ap))
            if qkv_bias:
                bqkt = singles.tile([128, FTOT], F32)
                nc.sync.dma_start(
                    bqkt[:],
                    bass.AP(tensor=bqk.tensor, offset=bqk[:].offset,
                            ap=[[0, 128]] + bqk[:].ap))

            # ---------------- P1 + P2 ----------------
            with (
                tc.tile_pool(name="p12sb", bufs=2) as p12,
                tc.tile_pool(name="p12w", bufs=1) as p12w,
                tc.tile_pool(name="p12ps", bufs=2, space="PSUM") as p12ps,
                tc.tile_pool(name="tps", bufs=2, space="PSUM") as tps,
            ):
                wqs = p12w.tile([128, 8, FTOT], MMDT)
                nc.gpsimd.dma_start(
                    wqs[:], wq[:].rearrange("(kc p) f -> p kc f", p=128))

                xTr = xT[:].rearrange("(kc p) l -> p kc l", p=128)

                for qtr in range(4):          # 4 token-tiles per quarter
                    t0 = qtr * 4
                    tok0 = t0 * 128

                    xq = p12.tile([128, 8, 512], MMDT, tag="xq")
                    nc.gpsimd.dma_start(xq[:], xTr[:, :, tok0 : tok0 + 512])
                    cq = p12.tile([128, 4, 8, 32], F32, tag="cq")
                    nc.sync.dma_start(cq[:], cosb[:, t0 : t0 + 4])
                    sq_ = p12.tile([128, 4, 8, 32], F32, tag="sq_")
                    nc.sync.dma_start(sq_[:], sinb[:, t0 : t0 + 4])

                    qkraw = p12.tile([128, 4, 8, DH], F32, tag="qkraw")

                    for t4 in range(4):
                        t = t0 + t4
                        qk_ps = p12ps.tile([128, FQK], F32, tag="qk_ps")
                        v_ps = p12ps.tile([128, FV], F32, tag="v_ps")
                        for kc in range(8):
                            lhsT = xq[:, kc, t4 * 128 : t4 * 128 + 128]
                            nc.tensor.matmul(
                                qk_ps[:], lhsT, wqs[:, kc, 0:FQK],
                                start=(kc == 0), stop=(kc == 7))
                        for kc in range(8):
                            lhsT = xq[:, kc, t4 * 128 : t4 * 128 + 128]
                            nc.tensor.matmul(
                                v_ps[:], lhsT, wqs[:, kc, FQK:FTOT],
                                start=(kc == 0), stop=(kc == 7))
                        if qkv_bias:
                            nc.vector.tensor_add(
                                qkraw[:, t4],
                                qk_ps[:].rearrange("p (g d) -> p g d", d=DH),
                                bqkt[:, 0:FQK].rearrange("p (g d) -> p g d", d=DH))
                            nc.vector.tensor_add(
                                vsb[:, t, :, 0:DH],
                                v_ps[:].rearrange("p (g d) -> p g d", d=DH),
                                bqkt[:, FQK:FTOT].rearrange("p (g d) -> p g d", d=DH))
                        else:
                            nc.scalar.copy(
                                qkraw[:, t4],
                                qk_ps[:].rearrange("p (g d) -> p g d", d=DH))
                            nc.scalar.copy(
                                vsb[:, t, :, 0:DH],
                                v_ps[:].rearrange("p (g d) -> p g d", d=DH))

                    if apply_gamma:
                        gview = bass.AP(
                            tensor=gamt.tensor, offset=gamt[:].offset,
                            ap=[gamt[:].ap[0], [0, 4]] + gamt[:].ap[1:])
                        nc.vector.tensor_mul(qkraw[:], qkraw[:], gview)

                    # RMS statistics
                    sqt = p12.tile([128, 4, 8, DH], F32, tag="qkrot")
                    nc.vector.tensor_mul(sqt[:], qkraw[:], qkraw[:])
                    ssq = p12.tile([128, 4, 8], F32, tag="ssq")
                    nc.vector.reduce_sum(ssq[:], sqt[:], axis=mybir.AxisListType.X)
                    nc.scalar.activation(
                        ssq[:], ssq[:], mybir.ActivationFunctionType.Sqrt,
                        bias=epst[:], scale=1.0 / DH)
                    nc.vector.reciprocal(ssq[:], ssq[:])

                    # RoPE
                    qkrot = p12.tile([128, 4, 8, DH], F32, tag="qkrot")
                    q1 = qkraw[:, :, :, 0:32]
                    q2 = qkraw[:, :, :, 32:64]
                    mA = p12.tile([128, 4, 8, 32], F32, tag="mA")
                    mB = p12.tile([128, 4, 8, 32], F32, tag="mB")
                    nc.vector.tensor_mul(mA[:], q1, cq[:])
                    nc.vector.tensor_mul(mB[:], q2, sq_[:])
                    nc.vector.tensor_sub(qkrot[:, :, :, 0:32], mA[:], mB[:])
                    mC = p12.tile([128, 4, 8, 32], F32, tag="mA")
                    mD = p12.tile([128, 4, 8, 32], F32, tag="mB")
                    nc.vector.tensor_mul(mC[:], q2, cq[:])
                    nc.vector.tensor_mul(mD[:], q1, sq_[:])
                    nc.vector.tensor_add(qkrot[:, :, :, 32:64], mC[:], mD[:])

                    # apply 1/rms (broadcast [128,4,8] over dh)
                    rview = bass.AP(
                        tensor=ssq.tensor, offset=ssq[:].offset,
                        ap=ssq[:].ap + [[0, DH]])
                    nc.vector.tensor_mul(qkrot[:], qkrot[:], rview)

                    # transpose pairs into qTs/kTs (f32 -> psum, cast on copy)
                    for t4 in range(4):
                        t = t0 + t4
                        for pr in range(4):
                            tp = tps.tile([128, 128], F32, tag="tp")
                            nc.tensor.transpose(
                                tp[:], qkrot[:, t4, 2 * pr : 2 * pr + 2, :],
                                ident[:])
                            dst = (qTs if pr < 2 else kTs)[pr % 2]
                            nc.vector.tensor_copy(
                                dst[:, t * 128 : (t + 1) * 128], tp[:])

            # ---------------- P3 + P4, per query half ----------------
            attnStack = [persist.tile([128, L], MMDT, tag=f"ast{i}", name=f"ast{i}")
                         for i in range(2)]
            den_d = dram.tile([2, HPC, 1024], F32)      # [half, head, q]
            rden_d = dram.tile([2, HPC, 1024], F32)     # reciprocals
            wos = persist.tile([128, 8, D], MMDT)
            nc.gpsimd.dma_start(
                wos[:], wo[:].rearrange("(kc p) o -> p kc o", p=128))

            # query-column offset of this core's shard within a half
            qreg = nc.sync.alloc_register("qoff_r")
            nc.sync.reg_load(qreg, qoff[0:1, 0:1])
            qv = nc.sync.snap(qreg, donate=True, min_val=0, max_val=768)

            with (
                tc.tile_pool(name="scps", bufs=2, space="PSUM") as scps,
                tc.tile_pool(name="atps", bufs=1, space="PSUM") as atps,
                tc.tile_pool(name="ops", bufs=2, space="PSUM") as ops,
                tc.tile_pool(name="pbsb", bufs=4) as pbsb,
                tc.tile_pool(name="recp", bufs=2) as recp,
                tc.tile_pool(name="denbp", bufs=2) as denbp,
                tc.tile_pool(name="osb", bufs=3) as osb,
                tc.tile_pool(name="agp", bufs=2) as agp,
            ):
                for half in range(2):
                    qlo = half * 1024
                    jmax = 8 if half == 0 else 16
                    for h in range(HPC):
                        kt = kTs[h // 2]
                        qt = qTs[h // 2]
                        pb0 = (h % 2) * 64
                        at = atps.tile([DH + 1, 1024], F32, tag="at")
                        for j in range(jmax):
                            wlo = max(j * 128, qlo)     # global query col start
                            w = qlo + 1024 - wlo
                            sc = scps.tile([128, 1024], F32, tag="sc")
                            for c0 in range(0, w, 512):
                                cw = min(512, w - c0)
                                nc.tensor.matmul(
                                    sc[:, c0 : c0 + cw],
                                    kt[pb0 : pb0 + 64, j * 128 : (j + 1) * 128],
                                    qt[pb0 : pb0 + 64, wlo + c0 : wlo + c0 + cw],
                                    start=True, stop=True)
                            pb = pbsb.tile([128, 1024], MMDT, tag="pb")
                            nc.scalar.activation(
                                pb[:, 0:w], sc[:, 0:w],
                                mybir.ActivationFunctionType.Exp, scale=1.0 / 8.0)
                            if half == 1 and j == 0:
                                # mask: last query frame can't see key frame 0
                                mask_ap = pb[:, 896:1024]
                                if MMDT == F32R:
                                    mask_ap = mask_ap.bitcast(F32)
                                nc.vector.memset(mask_ap, 0.0)
                            s_rel = wlo - qlo           # window start within half
                            for b0 in range(0, 1024, 512):
                                seg0 = max(s_rel, b0)
                                seg1 = b0 + 512
                                if seg0 >= seg1:
                                    continue
                                nc.tensor.matmul(
                                    at[:, seg0:seg1],
                                    vsb[:, j, h, :],
                                    pb[:, seg0 - s_rel : seg1 - s_rel],
                                    start=(j == 0),
                                    stop=(j == jmax - 1
                                          or (j + 1) * 128 >= qlo + seg1))
                        nc.vector.tensor_copy(
                            attnStack[h // 2][pb0 : pb0 + 64, qlo : qlo + 1024],
                            at[0:DH, :])
                        # stash the (un-reciprocated) denominator row
                        rec = recp.tile([DH + 1, 1024], F32, tag="rec")
                        nc.vector.tensor_copy(rec[DH : DH + 1, :],
                                              at[DH : DH + 1, :])
                        nc.gpsimd.dma_start(den_d[half, h, :],
                                            rec[DH : DH + 1, :])

                    # reciprocal on a [128, 32] repack (single-lane DVE
                    # reciprocal on [1,1024] rows costs ~6.5us each)
                    dpak = recp.tile([128, 32], F32, tag="dpak")
                    dh_ap = den_d[half]
                    nc.gpsimd.dma_start(
                        dpak[:],
                        bass.AP(tensor=dh_ap.tensor, offset=dh_ap.offset,
                                ap=[[32, 128], [1, 32]]))
                    nc.vector.reciprocal(dpak[:], dpak[:])
                    rh_ap = rden_d[half]
                    nc.gpsimd.dma_start(
                        bass.AP(tensor=rh_ap.tensor, offset=rh_ap.offset,
                                ap=[[32, 128], [1, 32]]),
                        dpak[:])

                    # normalize attnStack for this half
                    for pr in range(2):
                        denb = denbp.tile([128, 1024], F32, tag="denb")
                        for s in range(2):
                            src_ap = rden_d[half, 2 * pr + s, :]
                            nc.gpsimd.dma_start(
                                denb[s * 64 : (s + 1) * 64, :],
                                bass.AP(tensor=src_ap.tensor,
                                        offset=src_ap.offset,
                                        ap=[[0, 64]] + src_ap.ap))
                        nc.vector.tensor_mul(
                            attnStack[pr][:, qlo : qlo + 1024],
                            attnStack[pr][:, qlo : qlo + 1024],
                            denb[:])

                    # AllGather the (bf16) attention outputs across the
                    # 4-core group, then each core out-projects only its
                    # own 256-query shard with the full weight matrix.
                    ag_in = dram.tile([HPC * DH, 1024], MMDT, name=f"agin{half}")
                    for pr in range(2):
                        nc.sync.dma_start(
                            ag_in[pr * 128 : (pr + 1) * 128, :],
                            attnStack[pr][:, qlo : qlo + 1024])
                    ago = dram.tile([4, HPC * DH, 1024], MMDT, name=f"ago{half}")
                    nc.gpsimd.collective_compute(
                        "AllGather", mybir.AluOpType.bypass,
                        replica_groups=GROUPS,
                        ins=[ag_in[:].opt()], outs=[ago[:].opt()])
                    agsb = agp.tile([128, 4, 2, 256], MMDT, tag="agsb")
                    agov = ago[:].rearrange("s (pr p) q -> p s pr q", p=128)
                    nc.sync.dma_start(
                        agsb[:], agov[:, :, :, bass.ds(qv, 256)])

                    for qt in range(2):
                        ost = osb.tile([128, D], F32, tag="ost")
                        for o0 in range(0, D, 512):
                            op = ops.tile([128, 512], F32, tag="op")
                            for kc in range(8):
                                nc.tensor.matmul(
                                    op[:],
                                    agsb[:, kc // 2, kc % 2,
                                         qt * 128 : (qt + 1) * 128],
                                    wos[:, kc, o0 : o0 + 512],
                                    start=(kc == 0), stop=(kc == 7))
                            nc.vector.tensor_copy(ost[:, o0 : o0 + 512], op[:])
                        nc.sync.dma_start(
                            out[half * 256 + qt * 128 : half * 256 + (qt + 1) * 128, :],
                            ost[:])

    nc.compile()
    return nc


_PROG_CACHE = {}


def _get_program(key):
    if key not in _PROG_CACHE:
        _PROG_CACHE[key] = build_program(*key)
    return _PROG_CACHE[key]


def _host_inputs(x, W_qkv, b_qkv, W_out, b_out, q_gamma, k_gamma,
                 mm_dtype="bf16"):
    x = np.asarray(x, dtype=np.float32)
    W_qkv = np.asarray(W_qkv, dtype=np.float32)
    b_qkv = np.asarray(b_qkv, dtype=np.float32)
    W_out = np.asarray(W_out, dtype=np.float32)
    q_gamma = np.asarray(q_gamma, dtype=np.float32)
    k_gamma = np.asarray(k_gamma, dtype=np.float32)

    apply_gamma = not (np.all(q_gamma == 1.0) and np.all(k_gamma == 1.0))
    qkv_bias = bool(np.any(b_qkv))

    # rope tables: pos = t*128 + p
    pos = np.arange(L, dtype=np.float64).reshape(NT, 128).T  # [128, NT]
    inv = 1.0 / (ROPE_THETA ** (np.arange(32, dtype=np.float64) / 32.0))
    ang = pos[:, :, None] * inv[None, None, :]               # [128, NT, 32]
    cosb = np.broadcast_to(
        np.cos(ang)[:, :, None, :], (128, NT, 8, 32)).astype(np.float32).copy()
    sinb = np.broadcast_to(
        np.sin(ang)[:, :, None, :], (128, NT, 8, 32)).astype(np.float32).copy()

    Wq = W_qkv[0 * D : 1 * D]
    Wk = W_qkv[1 * D : 2 * D]
    Wv = W_qkv[2 * D : 3 * D]
    WoT = np.ascontiguousarray(W_out.T)  # [d_in, d_out]

    in_maps = []
    for c in range(N_CORES):
        b = c // 4
        h0 = 4 * (c % 4)
        rows = slice(h0 * DH, (h0 + HPC) * DH)
        wq_c = np.ascontiguousarray(
            np.concatenate([Wq[rows], Wk[rows], Wv[rows]], axis=0).T)
        m = {
            "xT": np.ascontiguousarray(x[b].T),
            "wq": wq_c,
            "wo": WoT,
            "cosb": cosb,
            "sinb": sinb,
            "qoff": np.array([[256 * (c % 4)]], dtype=np.uint32),
        }
        if apply_gamma:
            m["gam"] = np.ascontiguousarray(
                np.concatenate([np.broadcast_to(q_gamma, (4, DH)),
                                np.broadcast_to(k_gamma, (4, DH))], axis=0))
        if qkv_bias:
            m["bqk"] = np.ascontiguousarray(np.concatenate(
                [b_qkv[0 * D : 1 * D][rows], b_qkv[1 * D : 2 * D][rows],
                 b_qkv[2 * D : 3 * D][rows]]))
        in_maps.append(m)

    key = (mm_dtype, apply_gamma, qkv_bias)
    return key, in_maps


def _assemble(results, b_out):
    y = np.empty((B, L, D), dtype=np.float32)
    for c in range(N_CORES):
        b = c // 4
        r = c % 4
        o = results[c]["out"]
        for half in range(2):
            rows = slice(1024 * half + 256 * r, 1024 * half + 256 * r + 256)
            y[b, rows, :] = o[256 * half : 256 * half + 256]
    b_out = np.asarray(b_out, dtype=np.float32)
    if np.any(b_out):
        y += b_out
    return y


def _install_ntff_hook():
    """Register the axon NTFF profiling hook (the container's antenv stub
    lacks axon_hooks; replicate what trn_boot would have registered)."""
    import sys
    import types
    try:
        from antenv.axon_hooks import get_axon_ntff_profile_hook  # noqa: F401
        return
    except ImportError:
        pass
    try:
        from trn_agent_boot.trn_boot import _ntff_profile_via_ctypes
        hook = _ntff_profile_via_ctypes("/opt/axon/libaxon_pjrt.so")
    except Exception:
        hook = None
    import antenv
    mod = types.ModuleType("antenv.axon_hooks")
    mod.get_axon_ntff_profile_hook = lambda: hook
    mod.set_axon_ntff_profile_hook = lambda h: None
    antenv.axon_hooks = mod
    sys.modules["antenv.axon_hooks"] = mod


def kernel(x, W_qkv, b_qkv, W_out, b_out, q_gamma, k_gamma, _trace=False,
           _mm_dtype="bf16"):
    from concourse.bass_utils import run_bass_kernel_spmd
    if _trace:
        _install_ntff_hook()

    key, in_maps = _host_inputs(x, W_qkv, b_qkv, W_out, b_out,
                                q_gamma, k_gamma, _mm_dtype)
    nc = _get_program(key)
    res = run_bass_kernel_spmd(nc, in_maps, core_ids=list(range(N_CORES)),
                               trace=_trace,
                               trace_cores=list(range(N_CORES)) if _trace else None)
    y = _assemble(res.results, b_out)
    if _trace:
        return y, res
    return y
